# revision 52
# baseline (speedup 1.0000x reference)
"""DGCN diffusion-graph-conv kernel for 8 Trainium2 NeuronCores (v6).

Math (per the reference):
    support S = D^-1/2 (adj+I)^T D^-1/2,  D = diag(rowsum(adj+I))
    x_m = T_m(S) x0  (Chebyshev, K=3),  out = sum_m x_m @ W_m + bias

Folding the Chebyshev coefficients into the weights
    V0 = W0 - W2, V1 = W1 - 3*W3, V2 = 2*W2, V3 = 4*W3
gives out_b = sum_{m=0..3} S^m (X_b @ V_m).

With M = adj + I and d = rowsum(M)^-1/2, S^m factors as
    S^m = diag(d) (M^T d^2)^(m-1) M^T diag(d)
so defining G_m = M (d^2 M)^(m-1)  (G_1 = M), the apply stage is
    out[n,:] = U0[n,:] + d_n * sum_m sum_j G_m[j,n] * (d_j U_m[j,:]).
The outer diag(d) scales fold into the U eviction (d_j, a per-partition
scalar) and the final eviction (d_n); the powers G_2, G_3 need only
d^2 = 1/rowsum - a vector-engine reciprocal, keeping the whole matrix
pipeline off the scalar-engine sqrt path.

Per-core plan (data-parallel over batch, 4 batches/core):
    1.  M tiles in bf16 from adj; PE-transpose -> M^T; G2 = M d^2 M and
        G3 = M (d^2 M)^2 as bf16 matmuls.
    2.  U0 = X @ V0 + bias in bf16 matmuls (error-critical path).
    3.  U_m = X @ V_m (m=1..3) in fp8e4m3 DoubleRow (2 K-tiles/pass).
        Errors here are suppressed ~20x by the small S^m magnitudes.
    4.  out = U0 + d * (G-apply over stacked d*U) in fp8 DoubleRow.
fp8 scales: X x16, V x32, G_m x32, (d U_m) x128; descale d_n/2^12 is
applied per-partition in the final eviction.

Everything DMA-heavy ships in bf16 (inputs, adj, weights); dummy eye
matmuls warm the PE (HAM clock gate) before the real stream.
"""

import numpy as np
import ml_dtypes

import concourse.bacc as bacc
import concourse.tile as tile
import concourse.mybir as mybir
from concourse.bass_utils import run_bass_kernel_spmd

F32 = mybir.dt.float32
BF16 = mybir.dt.bfloat16
F8 = mybir.dt.float8e4
AX = mybir.AxisListType
ALU = mybir.AluOpType
DR = mybir.MatmulPerfMode.DoubleRow

N_CORES = 8
B, N, D = 32, 512, 768
BL = B // N_CORES          # batches per core = 4
BN = BL * N                # rows per core = 2048
NT = BN // 128             # 16 row blocks
DT = D // 128              # 6 feature tiles
DP = DT // 2               # 3 feature-tile pairs
JT = N // 128              # 4 node tiles
EC = 384                   # output-column chunk (psum-bank safe)

SX, SV, SB, SU = 16.0, 32.0, 32.0, 128.0
DU_EV = SU / (SX * SV)             # 1/4: psum(=512 U) -> d*U*128 with d AP
DESCALE = 1.0 / (SB * SU)          # 2^-12, folded with d_n into final AP

WARMUP_A = 24                      # bf16 junk matmuls to open the HAM gate


def _build_program():
    nc = bacc.Bacc("TRN2", target_bir_lowering=False, debug=False,
                   num_devices=N_CORES)
    inpT_d = nc.dram_tensor("inpT", [D, BN], BF16, kind="ExternalInput").ap()
    adj_d = nc.dram_tensor("adj8", [N, N], BF16, kind="ExternalInput").ap()
    adjT_d = nc.dram_tensor("adjT8", [N, N], BF16, kind="ExternalInput").ap()
    wts_d = nc.dram_tensor("wts16", [D * 4, D], BF16,
                           kind="ExternalInput").ap()
    bias_d = nc.dram_tensor("bias", [D], F32, kind="ExternalInput").ap()
    out_d = nc.dram_tensor("out", [BN, D], F32, kind="ExternalOutput").ap()

    wts_v = wts_d.rearrange("(d m) e -> m d e", m=4)

    with tile.TileContext(nc) as tc:
        with (
            tc.tile_pool(name="const", bufs=1) as constp,
            tc.tile_pool(name="x0", bufs=1) as x0p,
            tc.tile_pool(name="x8", bufs=1) as x8p,
            tc.tile_pool(name="wraw", bufs=8) as wp,
            tc.tile_pool(name="v0", bufs=1) as v0p,
            tc.tile_pool(name="v8", bufs=1) as v8p,
            tc.tile_pool(name="vtmp", bufs=6) as vtp,
            tc.tile_pool(name="sup", bufs=1) as supp,
            tc.tile_pool(name="pt8", bufs=1) as pt8p,
            tc.tile_pool(name="u0", bufs=1) as u0p,
            tc.tile_pool(name="u8", bufs=1) as u8p,
            tc.tile_pool(name="outst", bufs=4) as outp,
            tc.tile_pool(name="psA", bufs=8, space="PSUM") as psA,
        ):
            # ---- adj/adjT at the very head of the gpsimd ring ----
            adjts, adjTs = [], []
            for t in range(JT):
                a = supp.tile([128, N], BF16, name=f"adj{t}")
                nc.gpsimd.dma_start(a[:], adj_d[t * 128:(t + 1) * 128, :])
                adjts.append(a)
            for t in range(JT):
                aT = supp.tile([128, N], BF16, name=f"adjT{t}")
                nc.gpsimd.dma_start(aT[:], adjT_d[t * 128:(t + 1) * 128, :])
                adjTs.append(aT)
            bias_bc = constp.tile([128, D], F32)
            nc.gpsimd.dma_start(
                bias_bc[:], bias_d.unsqueeze(0).broadcast_to([128, D]))

            x0 = []   # filled in the support loop below (deferred issue)

            # ---- weights on sync queue: (W0,W2) pairs gate V0 -> U0 ----
            wtiles = {}
            worder = [(m, dt) for dt in range(DT) for m in (0, 2)] + \
                     [(m, dt) for dt in range(DT) for m in (1, 3)]
            for m, dt in worder:
                w = wp.tile([128, D], BF16, name=f"w{m}_{dt}", tag="wt")
                nc.sync.dma_start(
                    w[:], wts_v[m, dt * 128:(dt + 1) * 128, :])
                wtiles[(m, dt)] = w

            # ---- PE warmup on a memset tile: no DMA dependency, so the
            # HAM clock gate opens as soon as the program starts ----
            zw = constp.tile([128, 512], BF16)
            nc.vector.memzero(zw[:])
            for k in range(WARMUP_A):
                wps = psA.tile([128, 512], F32, name=f"wa{k}", tag="ps")
                nc.tensor.matmul(wps[:], zw[:, 0:128], zw[:],
                                 start=True, stop=True)

            # ------- rowsums, d^2, Mr = d^2 M  (per-tile pipeline).
            # adj8/adjT8 already carry the +I diagonal (host prep), so the
            # DMA tiles ARE M and M^T - no on-chip build needed. -------
            mbs, mts = adjts, adjTs
            dsqs, sqs, mrs = [], [], []
            for t in range(JT):
                rs = supp.tile([128, 1], F32, name=f"rs{t}")
                nc.vector.tensor_reduce(rs[:], adjts[t][:], axis=AX.X,
                                        op=ALU.add)
                dsq = supp.tile([128, 1], F32, name=f"dsq{t}")
                nc.vector.reciprocal(dsq[:], rs[:])
                sq = supp.tile([128, 1], F32, name=f"sq{t}")
                nc.scalar.sqrt(sq[:], rs[:])
                # x0 loads issue after the sqrts so their 3.1 MB does not
                # steal DMA bandwidth from adj in the critical first ~10us
                x = x0p.tile([128, BN], BF16, name=f"x0_{t}")
                nc.scalar.dma_start(x[:], inpT_d[t * 128:(t + 1) * 128, :])
                x0.append(x)
                dsqs.append(dsq)
                sqs.append(sq)
                mr = supp.tile([128, N], BF16, name=f"mr{t}")
                nc.vector.tensor_scalar_mul(mr[:], adjts[t][:], dsq[:])
                mrs.append(mr)
            for dt in range(JT, DT):
                x = x0p.tile([128, BN], BF16, name=f"x0_{dt}")
                nc.scalar.dma_start(x[:], inpT_d[dt * 128:(dt + 1) * 128, :])
                x0.append(x)

            # ---- V8 m=2 on scalar (early, frees W2); V0 subs on vector ----
            v8 = {}
            for m in (1, 2, 3):
                for dp in range(DP):
                    v8[(m, dp)] = v8p.tile([128, 2, D], F8,
                                           name=f"v8_{m}_{dp}")
            for dt in range(DT):
                nc.scalar.mul(v8[(2, dt // 2)][:, dt % 2, :],
                              wtiles[(2, dt)][:], 2.0 * SV)
            v0 = []
            for dt in range(DT):
                v = v0p.tile([128, D], BF16, name=f"v0_{dt}")
                nc.vector.tensor_sub(v[:], wtiles[(0, dt)][:],
                                     wtiles[(2, dt)][:])
                v0.append(v)
            vtmps = []
            for dt in range(DT):
                tmp = vtp.tile([128, D], BF16, name=f"vt_{dt}", tag="vt")
                nc.vector.scalar_tensor_tensor(
                    tmp[:], wtiles[(3, dt)][:], -3.0, wtiles[(1, dt)][:],
                    ALU.mult, ALU.add)
                vtmps.append(tmp)

            pt8 = {}
            for m in (1, 2, 3):
                for u in range(2):
                    pt8[(m, u)] = pt8p.tile([128, 2, N], F8,
                                            name=f"pt8_{m}_{u}")
            # ---- PT8 m=1 + d-column eviction scales (vector, low prio) ----
            for t in range(JT):
                nc.vector.tensor_scalar_mul(
                    pt8[(1, t // 2)][:, t % 2, :], mbs[t][:], SB)
            du_ev, dn_ev = [], []
            for t in range(JT):
                dcol = supp.tile([128, 1], F32, name=f"dcol{t}")
                nc.vector.reciprocal(dcol[:], sqs[t][:])
                du = supp.tile([128, 1], F32, name=f"du{t}")
                nc.vector.tensor_scalar_mul(du[:], dcol[:], DU_EV)
                dn = supp.tile([128, 1], F32, name=f"dn{t}")
                nc.vector.tensor_scalar_mul(dn[:], dcol[:], DESCALE)
                du_ev.append(du)
                dn_ev.append(dn)

            # ---------------- U0 = X @ V0 + bias (bf16) ----------------
            u0 = []
            for rb in range(NT):
                ut = u0p.tile([128, D], BF16, name=f"u0_{rb}")
                u0.append(ut)
                pss = [psA.tile([128, 512], F32, name=f"pu0_{rb}_{e}",
                                tag="ps") for e in range(2)]
                for dt in range(DT):
                    lhs = x0[dt][:, rb * 128:(rb + 1) * 128]
                    for e in range(2):
                        nc.tensor.matmul(
                            pss[e][:, 0:EC], lhs,
                            v0[dt][:, e * EC:(e + 1) * EC],
                            start=(dt == 0), stop=(dt == DT - 1))
                for e in range(2):
                    nc.vector.tensor_add(
                        ut[:, e * EC:(e + 1) * EC], pss[e][:, 0:EC],
                        bias_bc[:, e * EC:(e + 1) * EC])

            # ---------------- PE: G2 = M d2 M, G3 = M (d2 M)^2 ------------
            g2r = [supp.tile([128, N], BF16, name=f"g2r{t}")
                   for t in range(JT)]
            for ab in range(JT):
                ps = psA.tile([128, 512], F32, name=f"p2_{ab}", tag="ps")
                for cb in range(JT):
                    nc.tensor.matmul(
                        ps[:], mts[cb][:, ab * 128:(ab + 1) * 128],
                        mrs[cb][:], start=(cb == 0), stop=(cb == JT - 1))
                nc.scalar.mul(g2r[ab][:], ps[:], dsqs[ab][:])
                nc.scalar.mul(pt8[(2, ab // 2)][:, ab % 2, :], ps[:], SB)
            for ab in range(JT):
                ps = psA.tile([128, 512], F32, name=f"p3_{ab}", tag="ps")
                for cb in range(JT):
                    nc.tensor.matmul(
                        ps[:], mts[cb][:, ab * 128:(ab + 1) * 128],
                        g2r[cb][:], start=(cb == 0), stop=(cb == JT - 1))
                nc.scalar.mul(pt8[(3, ab // 2)][:, ab % 2, :], ps[:], SB)


            # ---- X8 quantize + V8 m=1,3 on scalar (needed by U123) ----
            x8 = []
            for dp in range(DP):
                t8 = x8p.tile([128, 2, BN], F8, name=f"x8_{dp}")
                for i in range(2):
                    nc.scalar.mul(t8[:, i, :], x0[2 * dp + i][:], SX)
                x8.append(t8)
            for dp in range(DP):
                for i in range(2):
                    dt = 2 * dp + i
                    nc.scalar.mul(v8[(1, dp)][:, i, :], vtmps[dt][:], SV)
                    nc.scalar.mul(v8[(3, dp)][:, i, :], wtiles[(3, dt)][:],
                                  4.0 * SV)

            # ---------------- U_m = X @ V_m (fp8 DoubleRow) ----------------
            u8 = {}
            for m in (1, 2, 3):
                for b in range(BL):
                    for u in range(2):
                        u8[(m, b, u)] = u8p.tile(
                            [128, 2, D], F8, name=f"u8_{m}_{b}_{u}")
            for rb in range(NT):
                b, jt = rb // JT, rb % JT
                u, i = jt // 2, jt % 2
                for m in (1, 2, 3):
                    pss = [psA.tile([128, 512], F32, name=f"pu{m}_{rb}_{e}",
                                    tag="ps") for e in range(2)]
                    for dp in range(DP):
                        lhs = x8[dp][:, :, rb * 128:(rb + 1) * 128]
                        for e in range(2):
                            nc.tensor.matmul(
                                pss[e][:, 0:EC], lhs,
                                v8[(m, dp)][:, :, e * EC:(e + 1) * EC],
                                start=(dp == 0), stop=(dp == DP - 1),
                                perf_mode=DR)
                    dst = u8[(m, b, u)]
                    ev = (nc.vector, nc.scalar)[(rb * 3 + m) % 2]
                    for e in range(2):
                        if ev is nc.scalar:
                            ev.mul(dst[:, i, e * EC:(e + 1) * EC],
                                   pss[e][:, 0:EC], du_ev[jt][:])
                        else:
                            ev.tensor_scalar_mul(
                                dst[:, i, e * EC:(e + 1) * EC],
                                pss[e][:, 0:EC], du_ev[jt][:])

            # ---------------- apply + final eviction ----------------
            MP = [(1, 0), (1, 1), (2, 0), (2, 1), (3, 0), (3, 1)]
            for b in range(BL):
                for nb in range(JT):
                    rb = b * JT + nb
                    pss = [psA.tile([128, 512], F32, name=f"pa_{rb}_{e}",
                                    tag="ps") for e in range(2)]
                    for k, (m, u) in enumerate(MP):
                        lhs = pt8[(m, u)][:, :, nb * 128:(nb + 1) * 128]
                        for e in range(2):
                            nc.tensor.matmul(
                                pss[e][:, 0:EC], lhs,
                                u8[(m, b, u)][:, :, e * EC:(e + 1) * EC],
                                start=(k == 0), stop=(k == len(MP) - 1),
                                perf_mode=DR)
                    so = outp.tile([128, D], F32, name=f"so_{rb}", tag="so")
                    for e in range(2):
                        nc.vector.scalar_tensor_tensor(
                            so[:, e * EC:(e + 1) * EC], pss[e][:, 0:EC],
                            dn_ev[nb][:], u0[rb][:, e * EC:(e + 1) * EC],
                            ALU.mult, ALU.add)
                    nc.sync.dma_start(
                        out_d[rb * 128:(rb + 1) * 128, :], so[:])
    nc.compile()
    return nc


_CACHE = {}


def _get_program():
    if "nc" not in _CACHE:
        _CACHE["nc"] = _build_program()
    return _CACHE["nc"]


def make_in_maps(inputs, adj, weights, biases):
    inputs = np.ascontiguousarray(inputs, dtype=np.float32)
    adj = np.ascontiguousarray(adj, dtype=np.float32)
    weights = np.ascontiguousarray(weights, dtype=np.float32)
    biases = np.ascontiguousarray(biases, dtype=np.float32)
    assert inputs.shape == (B, N, D)
    assert adj.shape == (N, N)
    assert weights.shape == (D * 4, D)
    assert biases.shape == (D,)
    m_full = adj + np.eye(N, dtype=np.float32)
    adj8 = m_full.astype(ml_dtypes.bfloat16)
    adjT8 = np.ascontiguousarray(m_full.T).astype(ml_dtypes.bfloat16)
    wts16 = weights.astype(ml_dtypes.bfloat16)
    in_maps = []
    for c in range(N_CORES):
        x0T = np.ascontiguousarray(
            inputs[c * BL:(c + 1) * BL].reshape(BN, D).T).astype(
                ml_dtypes.bfloat16)
        in_maps.append({
            "inpT": x0T,
            "adj8": adj8,
            "adjT8": adjT8,
            "wts16": wts16,
            "bias": biases,
        })
    return in_maps


def kernel(inputs, adj, weights, biases):
    nc = _get_program()
    in_maps = make_in_maps(inputs, adj, weights, biases)
    res = run_bass_kernel_spmd(nc, in_maps, list(range(N_CORES)))
    out = np.concatenate(
        [res.results[c]["out"].reshape(BL, N, D) for c in range(N_CORES)],
        axis=0)
    return out


# revision 53
# speedup vs baseline: 1.0301x; 1.0301x over previous
"""DGCN diffusion-graph-conv kernel for 8 Trainium2 NeuronCores (v6).

Math (per the reference):
    support S = D^-1/2 (adj+I)^T D^-1/2,  D = diag(rowsum(adj+I))
    x_m = T_m(S) x0  (Chebyshev, K=3),  out = sum_m x_m @ W_m + bias

Folding the Chebyshev coefficients into the weights
    V0 = W0 - W2, V1 = W1 - 3*W3, V2 = 2*W2, V3 = 4*W3
gives out_b = sum_{m=0..3} S^m (X_b @ V_m).

With M = adj + I and d = rowsum(M)^-1/2, S^m factors as
    S^m = diag(d) (M^T d^2)^(m-1) M^T diag(d)
so defining G_m = M (d^2 M)^(m-1)  (G_1 = M), the apply stage is
    out[n,:] = U0[n,:] + d_n * sum_m sum_j G_m[j,n] * (d_j U_m[j,:]).
The outer diag(d) scales fold into the U eviction (d_j, a per-partition
scalar) and the final eviction (d_n); the powers G_2, G_3 need only
d^2 = 1/rowsum - a vector-engine reciprocal, keeping the whole matrix
pipeline off the scalar-engine sqrt path.

Per-core plan (data-parallel over batch, 4 batches/core):
    1.  M tiles in bf16 from adj; PE-transpose -> M^T; G2 = M d^2 M and
        G3 = M (d^2 M)^2 as bf16 matmuls.
    2.  U0 = X @ V0 + bias in bf16 matmuls (error-critical path).
    3.  U_m = X @ V_m (m=1..3) in fp8e4m3 DoubleRow (2 K-tiles/pass).
        Errors here are suppressed ~20x by the small S^m magnitudes.
    4.  out = U0 + d * (G-apply over stacked d*U) in fp8 DoubleRow.
fp8 scales: X x16, V x32, G_m x32, (d U_m) x128; descale d_n/2^12 is
applied per-partition in the final eviction.

Everything DMA-heavy ships in bf16 (inputs, adj, weights); dummy eye
matmuls warm the PE (HAM clock gate) before the real stream.
"""

import numpy as np
import ml_dtypes

import concourse.bacc as bacc
import concourse.tile as tile
import concourse.mybir as mybir
from concourse.bass_utils import run_bass_kernel_spmd

F32 = mybir.dt.float32
BF16 = mybir.dt.bfloat16
F8 = mybir.dt.float8e4
AX = mybir.AxisListType
ALU = mybir.AluOpType
DR = mybir.MatmulPerfMode.DoubleRow

N_CORES = 8
B, N, D = 32, 512, 768
BL = B // N_CORES          # batches per core = 4
BN = BL * N                # rows per core = 2048
NT = BN // 128             # 16 row blocks
DT = D // 128              # 6 feature tiles
DP = DT // 2               # 3 feature-tile pairs
JT = N // 128              # 4 node tiles
EC = 384                   # output-column chunk (psum-bank safe)

SX, SV, SB, SU = 16.0, 32.0, 32.0, 128.0
DU_EV = SU / (SX * SV)             # 1/4: psum(=512 U) -> d*U*128 with d AP
DESCALE = 1.0 / (SB * SU)          # 2^-12, folded with d_n into final AP

WARMUP_A = 24                      # bf16 junk matmuls to open the HAM gate


def _build_program():
    nc = bacc.Bacc("TRN2", target_bir_lowering=False, debug=False,
                   num_devices=N_CORES)
    inpT_d = nc.dram_tensor("inpT", [D, BN], BF16, kind="ExternalInput").ap()
    adj_d = nc.dram_tensor("adj8", [N, N], BF16, kind="ExternalInput").ap()
    adjT_d = nc.dram_tensor("adjT8", [N, N], BF16, kind="ExternalInput").ap()
    wts_d = nc.dram_tensor("wts16", [D * 4, D], BF16,
                           kind="ExternalInput").ap()
    bias_d = nc.dram_tensor("bias", [D], F32, kind="ExternalInput").ap()
    out_d = nc.dram_tensor("out", [BN, D], F32, kind="ExternalOutput").ap()

    wts_v = wts_d.rearrange("(d m) e -> m d e", m=4)

    with tile.TileContext(nc) as tc:
        with (
            tc.tile_pool(name="const", bufs=1) as constp,
            tc.tile_pool(name="x0", bufs=1) as x0p,
            tc.tile_pool(name="x8", bufs=1) as x8p,
            tc.tile_pool(name="wraw", bufs=8) as wp,
            tc.tile_pool(name="v0", bufs=1) as v0p,
            tc.tile_pool(name="v8", bufs=1) as v8p,
            tc.tile_pool(name="vtmp", bufs=6) as vtp,
            tc.tile_pool(name="sup", bufs=1) as supp,
            tc.tile_pool(name="pt8", bufs=1) as pt8p,
            tc.tile_pool(name="u0", bufs=1) as u0p,
            tc.tile_pool(name="u8", bufs=1) as u8p,
            tc.tile_pool(name="outst", bufs=4) as outp,
            tc.tile_pool(name="psA", bufs=8, space="PSUM") as psA,
        ):
            # ---- adj/adjT at the very head of the gpsimd ring ----
            adjts, adjTs = [], []
            for t in range(JT):
                a = supp.tile([128, N], BF16, name=f"adj{t}")
                nc.gpsimd.dma_start(a[:], adj_d[t * 128:(t + 1) * 128, :])
                adjts.append(a)
            for t in range(JT):
                aT = supp.tile([128, N], BF16, name=f"adjT{t}")
                nc.gpsimd.dma_start(aT[:], adjT_d[t * 128:(t + 1) * 128, :])
                adjTs.append(aT)
            bias_bc = constp.tile([128, D], F32)
            nc.gpsimd.dma_start(
                bias_bc[:], bias_d.unsqueeze(0).broadcast_to([128, D]))

            x0 = []   # filled in the support loop below (deferred issue)

            # ---- weights on sync queue: (W0,W2) pairs gate V0 -> U0 ----
            wtiles = {}
            worder = [(m, dt) for dt in range(DT) for m in (0, 2)] + \
                     [(m, dt) for dt in range(DT) for m in (1, 3)]
            for m, dt in worder:
                w = wp.tile([128, D], BF16, name=f"w{m}_{dt}", tag="wt")
                nc.sync.dma_start(
                    w[:], wts_v[m, dt * 128:(dt + 1) * 128, :])
                wtiles[(m, dt)] = w

            # ---- PE warmup on a memset tile: no DMA dependency, so the
            # HAM clock gate opens as soon as the program starts ----
            zw = constp.tile([128, 512], BF16)
            nc.vector.memzero(zw[:])
            for k in range(WARMUP_A):
                wps = psA.tile([128, 512], F32, name=f"wa{k}", tag="ps")
                nc.tensor.matmul(wps[:], zw[:, 0:128], zw[:],
                                 start=True, stop=True)

            # ------- rowsums, d^2, Mr = d^2 M  (per-tile pipeline).
            # adj8/adjT8 already carry the +I diagonal (host prep), so the
            # DMA tiles ARE M and M^T - no on-chip build needed. -------
            mbs, mts = adjts, adjTs
            dsqs, sqs, mrs = [], [], []
            for t in range(JT):
                rs = supp.tile([128, 1], F32, name=f"rs{t}")
                nc.vector.tensor_reduce(rs[:], adjts[t][:], axis=AX.X,
                                        op=ALU.add)
                dsq = supp.tile([128, 1], F32, name=f"dsq{t}")
                nc.vector.reciprocal(dsq[:], rs[:])
                sq = supp.tile([128, 1], F32, name=f"sq{t}")
                nc.scalar.sqrt(sq[:], rs[:])
                # x0 loads issue after the sqrts so their 3.1 MB does not
                # steal DMA bandwidth from adj in the critical first ~10us
                x = x0p.tile([128, BN], BF16, name=f"x0_{t}")
                nc.scalar.dma_start(x[:], inpT_d[t * 128:(t + 1) * 128, :])
                x0.append(x)
                dsqs.append(dsq)
                sqs.append(sq)
                mr = supp.tile([128, N], BF16, name=f"mr{t}")
                nc.vector.tensor_scalar_mul(mr[:], adjts[t][:], dsq[:])
                mrs.append(mr)
            for dt in range(JT, DT):
                x = x0p.tile([128, BN], BF16, name=f"x0_{dt}")
                nc.scalar.dma_start(x[:], inpT_d[dt * 128:(dt + 1) * 128, :])
                x0.append(x)

            # ---------------- PE: G2 = M d2 M, G3 = M (d2 M)^2 ------------
            pt8 = {}
            for m in (1, 2, 3):
                for u in range(2):
                    pt8[(m, u)] = pt8p.tile([128, 2, N], F8,
                                            name=f"pt8_{m}_{u}")
            g2r = [supp.tile([128, N], BF16, name=f"g2r{t}")
                   for t in range(JT)]
            for ab in range(JT):
                ps = psA.tile([128, 512], F32, name=f"p2_{ab}", tag="ps")
                for cb in range(JT):
                    nc.tensor.matmul(
                        ps[:], mts[cb][:, ab * 128:(ab + 1) * 128],
                        mrs[cb][:], start=(cb == 0), stop=(cb == JT - 1))
                nc.scalar.mul(g2r[ab][:], ps[:], dsqs[ab][:])
                nc.scalar.mul(pt8[(2, ab // 2)][:, ab % 2, :], ps[:], SB)
            for ab in range(JT):
                ps = psA.tile([128, 512], F32, name=f"p3_{ab}", tag="ps")
                for cb in range(JT):
                    nc.tensor.matmul(
                        ps[:], mts[cb][:, ab * 128:(ab + 1) * 128],
                        g2r[cb][:], start=(cb == 0), stop=(cb == JT - 1))
                nc.scalar.mul(pt8[(3, ab // 2)][:, ab % 2, :], ps[:], SB)

            # ---- V8 m=2 on scalar (early, frees W2); V0 subs on vector ----
            v8 = {}
            for m in (1, 2, 3):
                for dp in range(DP):
                    v8[(m, dp)] = v8p.tile([128, 2, D], F8,
                                           name=f"v8_{m}_{dp}")
            for dt in range(DT):
                nc.scalar.mul(v8[(2, dt // 2)][:, dt % 2, :],
                              wtiles[(2, dt)][:], 2.0 * SV)
            v0 = []
            for dt in range(DT):
                v = v0p.tile([128, D], BF16, name=f"v0_{dt}")
                nc.vector.tensor_sub(v[:], wtiles[(0, dt)][:],
                                     wtiles[(2, dt)][:])
                v0.append(v)
            vtmps = []
            for dt in range(DT):
                tmp = vtp.tile([128, D], BF16, name=f"vt_{dt}", tag="vt")
                nc.vector.scalar_tensor_tensor(
                    tmp[:], wtiles[(3, dt)][:], -3.0, wtiles[(1, dt)][:],
                    ALU.mult, ALU.add)
                vtmps.append(tmp)

            # ---- PT8 m=1 + d-column eviction scales (vector, low prio) ----
            for t in range(JT):
                nc.vector.tensor_scalar_mul(
                    pt8[(1, t // 2)][:, t % 2, :], mbs[t][:], SB)
            du_ev, dn_ev = [], []
            for t in range(JT):
                dcol = supp.tile([128, 1], F32, name=f"dcol{t}")
                nc.vector.reciprocal(dcol[:], sqs[t][:])
                du = supp.tile([128, 1], F32, name=f"du{t}")
                nc.vector.tensor_scalar_mul(du[:], dcol[:], DU_EV)
                dn = supp.tile([128, 1], F32, name=f"dn{t}")
                nc.vector.tensor_scalar_mul(dn[:], dcol[:], DESCALE)
                du_ev.append(du)
                dn_ev.append(dn)

            # ---------------- U0 = X @ V0 + bias (bf16) ----------------
            u0 = []
            for rb in range(NT):
                ut = u0p.tile([128, D], BF16, name=f"u0_{rb}")
                u0.append(ut)
                pss = [psA.tile([128, 512], F32, name=f"pu0_{rb}_{e}",
                                tag="ps") for e in range(2)]
                for dt in range(DT):
                    lhs = x0[dt][:, rb * 128:(rb + 1) * 128]
                    for e in range(2):
                        nc.tensor.matmul(
                            pss[e][:, 0:EC], lhs,
                            v0[dt][:, e * EC:(e + 1) * EC],
                            start=(dt == 0), stop=(dt == DT - 1))
                for e in range(2):
                    nc.vector.tensor_add(
                        ut[:, e * EC:(e + 1) * EC], pss[e][:, 0:EC],
                        bias_bc[:, e * EC:(e + 1) * EC])

            # ---- X8 quantize + V8 m=1,3 on scalar (needed by U123) ----
            x8 = []
            for dp in range(DP):
                t8 = x8p.tile([128, 2, BN], F8, name=f"x8_{dp}")
                for i in range(2):
                    nc.scalar.mul(t8[:, i, :], x0[2 * dp + i][:], SX)
                x8.append(t8)
            for dp in range(DP):
                for i in range(2):
                    dt = 2 * dp + i
                    nc.scalar.mul(v8[(1, dp)][:, i, :], vtmps[dt][:], SV)
                    nc.scalar.mul(v8[(3, dp)][:, i, :], wtiles[(3, dt)][:],
                                  4.0 * SV)

            # ---------------- U_m = X @ V_m (fp8 DoubleRow) ----------------
            u8 = {}
            for m in (1, 2, 3):
                for b in range(BL):
                    for u in range(2):
                        u8[(m, b, u)] = u8p.tile(
                            [128, 2, D], F8, name=f"u8_{m}_{b}_{u}")
            for rb in range(NT):
                b, jt = rb // JT, rb % JT
                u, i = jt // 2, jt % 2
                for m in (1, 2, 3):
                    pss = [psA.tile([128, 512], F32, name=f"pu{m}_{rb}_{e}",
                                    tag="ps") for e in range(2)]
                    for dp in range(DP):
                        lhs = x8[dp][:, :, rb * 128:(rb + 1) * 128]
                        for e in range(2):
                            nc.tensor.matmul(
                                pss[e][:, 0:EC], lhs,
                                v8[(m, dp)][:, :, e * EC:(e + 1) * EC],
                                start=(dp == 0), stop=(dp == DP - 1),
                                perf_mode=DR)
                    dst = u8[(m, b, u)]
                    ev = (nc.vector, nc.scalar)[(rb * 3 + m) % 2]
                    for e in range(2):
                        if ev is nc.scalar:
                            ev.mul(dst[:, i, e * EC:(e + 1) * EC],
                                   pss[e][:, 0:EC], du_ev[jt][:])
                        else:
                            ev.tensor_scalar_mul(
                                dst[:, i, e * EC:(e + 1) * EC],
                                pss[e][:, 0:EC], du_ev[jt][:])

            # ---------------- apply + final eviction ----------------
            MP = [(1, 0), (1, 1), (2, 0), (2, 1), (3, 0), (3, 1)]
            for b in range(BL):
                for nb in range(JT):
                    rb = b * JT + nb
                    pss = [psA.tile([128, 512], F32, name=f"pa_{rb}_{e}",
                                    tag="ps") for e in range(2)]
                    for k, (m, u) in enumerate(MP):
                        lhs = pt8[(m, u)][:, :, nb * 128:(nb + 1) * 128]
                        for e in range(2):
                            nc.tensor.matmul(
                                pss[e][:, 0:EC], lhs,
                                u8[(m, b, u)][:, :, e * EC:(e + 1) * EC],
                                start=(k == 0), stop=(k == len(MP) - 1),
                                perf_mode=DR)
                    so = outp.tile([128, D], F32, name=f"so_{rb}", tag="so")
                    for e in range(2):
                        nc.vector.scalar_tensor_tensor(
                            so[:, e * EC:(e + 1) * EC], pss[e][:, 0:EC],
                            dn_ev[nb][:], u0[rb][:, e * EC:(e + 1) * EC],
                            ALU.mult, ALU.add)
                    nc.sync.dma_start(
                        out_d[rb * 128:(rb + 1) * 128, :], so[:])
    nc.compile()
    return nc


_CACHE = {}


def _get_program():
    if "nc" not in _CACHE:
        _CACHE["nc"] = _build_program()
    return _CACHE["nc"]


def make_in_maps(inputs, adj, weights, biases):
    inputs = np.ascontiguousarray(inputs, dtype=np.float32)
    adj = np.ascontiguousarray(adj, dtype=np.float32)
    weights = np.ascontiguousarray(weights, dtype=np.float32)
    biases = np.ascontiguousarray(biases, dtype=np.float32)
    assert inputs.shape == (B, N, D)
    assert adj.shape == (N, N)
    assert weights.shape == (D * 4, D)
    assert biases.shape == (D,)
    m_full = adj + np.eye(N, dtype=np.float32)
    adj8 = m_full.astype(ml_dtypes.bfloat16)
    adjT8 = np.ascontiguousarray(m_full.T).astype(ml_dtypes.bfloat16)
    wts16 = weights.astype(ml_dtypes.bfloat16)
    in_maps = []
    for c in range(N_CORES):
        x0T = np.ascontiguousarray(
            inputs[c * BL:(c + 1) * BL].reshape(BN, D).T).astype(
                ml_dtypes.bfloat16)
        in_maps.append({
            "inpT": x0T,
            "adj8": adj8,
            "adjT8": adjT8,
            "wts16": wts16,
            "bias": biases,
        })
    return in_maps


def kernel(inputs, adj, weights, biases):
    nc = _get_program()
    in_maps = make_in_maps(inputs, adj, weights, biases)
    res = run_bass_kernel_spmd(nc, in_maps, list(range(N_CORES)))
    out = np.concatenate(
        [res.results[c]["out"].reshape(BL, N, D) for c in range(N_CORES)],
        axis=0)
    return out
